# revision 38
# baseline (speedup 1.0000x reference)
"""Causal multi-head attention on 8 Trainium2 NeuronCores.

Problem: x[2,4096,512], W_q/W_k/W_v/W_proj[512,512], b_proj[512]
  q,k,v = x @ W.T split into 8 heads of 64; causal softmax(q k^T / 8) v;
  out = attn @ W_proj.T + b_proj.

Sharding: 16 (batch, head) pairs over 8 cores -> each core gets one batch
and a pair of adjacent heads (128 of the 512 hidden dims).  The output
projection is computed per-core against the matching 128-row slice of
W_proj^T, giving a partial [4096, 512] output per core; the host sums the
4 partials per batch and adds the bias.

v3 design (bf16 value path + multi-engine softmax):
  - all matmul operands bf16 (fp8 was tried: its quantization noise on
    q/k/ex/v passes ~1:1 into the output and blows the 2e-2 budget)
  - q/k projections -> PSUM f32 -> bf16 qTb/kTb (Act/DVE copies)
  - scores: plain bf16 matmuls, one [128,512] block each (512 PE cycles)
  - exp(s/8 - 3) split between two engines (Pool cannot read PSUM):
      Act: native Exp -> bf16 (ex tile is int16; bf16 written via bitcast)
      DVE: Schraudolph int16 trick -> bf16 bits (one tensor_scalar)
  - causal masking post-exp on the diagonal pairs: one fused [128,2,256]
    multiply per pair (DVE or Pool on the bf16 view)
  - attnV: per k-block matmul with vb = [64 v cols | 64 ones cols], so acc
    rows 64..127 accumulate the softmax denominator broadcast across 64
    partitions; column-restricted for diagonal blocks
  - normalize: rf = 1/acc[64:128] (DVE), attnT = acc[0:64] * rf (DVE)
  - output projection bf16; PSUM -> bf16 partial out (Act/DVE copies)
  - proj / outproj / normalize emitted via a deferred work-queue drained
    between attention pairs, so the in-order engine queues never bunch
"""

import numpy as np

B, S, D, H = 2, 4096, 512, 8
DH = 64
QCHUNK = 512
SCALE = 1.0 / np.sqrt(DH)
ESHIFT = -3.0           # logit shift: exp(s/8 - 3); cancels in normalize
# int16 Schraudolph: bf16(bits) ~ exp(s/8 - 3), bits = A*s + B (truncated)
FEXP_A = 128 * 0.125 / np.log(2)          # 23.0831
FEXP_B = 15700.5                          # tuned numerically (trunc)

# exp engine weights (Act, DVE); Pool has no PSUM access
W_ACT, W_DVE = 0.52, 0.48
# PSUM-draining copy weights (Act, DVE)
WO_ACT, WO_DVE = 0.85, 0.15
# mask engine weights (DVE, Pool)
WM_DVE, WM_POOL = 0.25, 0.75

_CACHE = {}


class _WeightedRR:
    def __init__(self, weights: dict):
        self.w = dict(weights)
        self.acc = {k: 0.0 for k in weights}

    def next(self):
        for k in self.w:
            self.acc[k] += self.w[k]
        k = max(self.acc, key=lambda x: self.acc[x])
        self.acc[k] -= 1.0
        return k


def _build(s=S, normalize=True, repeats=1):
    from contextlib import ExitStack

    import concourse.mybir as mybir
    import concourse.tile as tile
    from concourse import bacc

    f32 = mybir.dt.float32
    bf16 = mybir.dt.bfloat16

    nkb_all = s // 128     # k blocks
    ndc = D // 128         # D chunks (contraction for projections)

    nc = bacc.Bacc("TRN2")
    xT_d = nc.dram_tensor("xT", [D, s], bf16, kind="ExternalInput")
    wqT_d = nc.dram_tensor("wqT", [D, 128], bf16, kind="ExternalInput")
    wkT_d = nc.dram_tensor("wkT", [D, 128], bf16, kind="ExternalInput")
    wvT_d = nc.dram_tensor("wvT", [D, 128], bf16, kind="ExternalInput")
    wpT_d = nc.dram_tensor("wpT", [128, D], bf16, kind="ExternalInput")
    out_d = nc.dram_tensor("out_p", [s, D], bf16, kind="ExternalOutput")

    with ExitStack() as ctx:
        tc = ctx.enter_context(tile.TileContext(nc))
        consts = ctx.enter_context(tc.tile_pool(name="consts", bufs=1))
        big = ctx.enter_context(tc.tile_pool(name="big", bufs=1))
        expool = ctx.enter_context(tc.tile_pool(name="expool", bufs=6))
        recpool = ctx.enter_context(tc.tile_pool(name="recpool", bufs=2))
        outpool = ctx.enter_context(tc.tile_pool(name="outpool", bufs=3))
        mmps = ctx.enter_context(tc.tile_pool(name="mmps", bufs=2, space="PSUM"))
        scps = ctx.enter_context(tc.tile_pool(name="scps", bufs=5, space="PSUM"))
        accps = ctx.enter_context(tc.tile_pool(name="accps", bufs=1, space="PSUM"))

        # ---- persistent SBUF ----
        xT = [big.tile([128, s], bf16, name=f"xT{c}", tag=f"xT{c}")
              for c in range(ndc)]
        qTb = big.tile([128, s], bf16, name="qTb", tag="qTb")
        kTb = big.tile([128, s], bf16, name="kTb", tag="kTb")
        # per k-block: [128 cols head0][128 cols head1]; each 128-col block
        # is [64 value cols | 64 ones cols] so the attnV output rows 64..127
        # accumulate the softmax denominator, pre-broadcast over partitions
        vb = big.tile([128, 256 * nkb_all], bf16, name="vb", tag="vb")
        attnT = big.tile([128, s], bf16, name="attnT", tag="attnT")
        wq = consts.tile([128, D], bf16, name="wq", tag="wq")
        wk = consts.tile([128, D], bf16, name="wk", tag="wk")
        wv = consts.tile([128, D], bf16, name="wv", tag="wv")
        wp = consts.tile([128, D], bf16, name="wp", tag="wp")
        maskD = consts.tile([128, 512], f32, name="maskD", tag="maskD")
        nbias = consts.tile([128, 1], f32, name="nbias", tag="nbias")

        for _rep in range(repeats):
            _emit_body(nc, tc, locals())

    nc.compile()
    return nc


def _emit_body(nc, tc, env):
    """One full pass of the kernel body (DMAs + all chunks)."""
    import concourse.mybir as mybir

    f32 = mybir.dt.float32
    bf16 = mybir.dt.bfloat16
    i16 = mybir.dt.int16
    EXP = mybir.ActivationFunctionType.Exp
    GE = mybir.AluOpType.is_ge
    MUL = mybir.AluOpType.mult
    ADD = mybir.AluOpType.add

    (s, nkb_all, ndc, normalize) = (
        env["s"], env["nkb_all"], env["ndc"], env["normalize"])
    (xT_d, wqT_d, wkT_d, wvT_d, wpT_d, out_d) = (
        env["xT_d"], env["wqT_d"], env["wkT_d"], env["wvT_d"], env["wpT_d"],
        env["out_d"])
    (xT, qTb, kTb, vb, attnT, wq, wk, wv, wp, maskD, nbias) = (
        env["xT"], env["qTb"], env["kTb"], env["vb"], env["attnT"],
        env["wq"], env["wk"], env["wv"], env["wp"], env["maskD"],
        env["nbias"])
    (consts, big, expool, recpool, outpool, mmps, scps, accps) = (
        env["consts"], env["big"], env["expool"], env["recpool"],
        env["outpool"], env["mmps"], env["scps"], env["accps"])
    nqc = s // QCHUNK

    exp_rr = _WeightedRR({"A": W_ACT, "D": W_DVE})
    out_rr = _WeightedRR({"A": WO_ACT, "D": WO_DVE})
    mask_rr = _WeightedRR({"D": WM_DVE, "P": WM_POOL})

    # ---- input DMAs + consts ----
    for w_sb, w_d, eng in ((wq, wqT_d, nc.sync), (wk, wkT_d, nc.scalar),
                           (wv, wvT_d, nc.scalar)):
        eng.dma_start(
            out=w_sb.rearrange("p (c m) -> p c m", c=ndc),
            in_=w_d.rearrange("(c p) m -> p c m", c=ndc))
    # chunks 0-2 as small pieces (fast start), the rest as one big DMA
    for qc in range(3):
        for c in range(ndc):
            cs = slice(qc * QCHUNK, (qc + 1) * QCHUNK)
            nc.sync.dma_start(out=xT[c][:, cs],
                              in_=xT_d[c * 128:(c + 1) * 128, cs])
    for c in range(ndc):
        cs = slice(3 * QCHUNK, nqc * QCHUNK)
        nc.sync.dma_start(out=xT[c][:, cs],
                          in_=xT_d[c * 128:(c + 1) * 128, cs])
    nc.sync.dma_start(out=wp, in_=wpT_d.ap())
    # ones half-blocks of vb (cols 64..127 of each 128-col block)
    ones_ap = vb.rearrange("p (k c) -> p k c", c=128)[:, :, 64:128]
    nc.gpsimd.memset(ones_ap, 1.0)
    nc.gpsimd.memset(nbias, float(ESHIFT))
    # maskD: cols 0-255: 1[f >= p]; cols 256-511: 1[f-256 >= 128+p]
    nc.gpsimd.memset(maskD, 1.0)
    nc.gpsimd.affine_select(
        out=maskD[:, 0:256], in_=maskD[:, 0:256], compare_op=GE, fill=0.0,
        base=0, channel_multiplier=-1, pattern=[[1, 256]])
    nc.gpsimd.affine_select(
        out=maskD[:, 256:512], in_=maskD[:, 256:512], compare_op=GE, fill=0.0,
        base=-128, channel_multiplier=-1, pattern=[[1, 256]])

    # Deferred PE work-groups (projection / output-projection / normalize).
    # Drained between attention pairs so the 1-buf "mm" PSUM ring round-trips
    # hide under the exp stream and engine queues never bunch at boundaries.
    pe_queue = []  # items: (kind, chunk, emit_fn)

    def drain_pe(n=1):
        for _ in range(n):
            if pe_queue:
                pe_queue.pop(0)[2]()

    def drain_proj_through(qc):
        # everything attention(qc) reads (qTb/kTb/vb of chunks <= qc) must
        # be EMITTED before the first score matmul reads it -- the Tile
        # framework orders later-emitted writers AFTER readers (WAR)
        while any(k == "proj" and c <= qc for k, c, _ in pe_queue):
            drain_pe()

    def psum_drain_copy(dst, src):
        if out_rr.next() == "A":
            nc.scalar.copy(dst, src)
        else:
            nc.vector.tensor_copy(dst, src)

    def queue_proj(qc):
        qlo = qc * QCHUNK
        qs = slice(qlo, qlo + QCHUNK)

        def qk_group(w_sb, dst):
            def emit():
                ps = mmps.tile([128, QCHUNK], f32,
                               name=f"proj{qc}_{dst.name}", tag="mm")
                for c in range(ndc):
                    nc.tensor.matmul(ps,
                                     lhsT=w_sb[:, c * 128:(c + 1) * 128],
                                     rhs=xT[c][:, qs],
                                     start=(c == 0), stop=(c == ndc - 1))
                psum_drain_copy(dst[:, qs], ps)
            return emit

        def v_group(kb):
            def emit():
                vp = mmps.tile([128, 128], f32, name=f"vp_{kb}", tag="mm")
                for c in range(ndc):
                    nc.tensor.matmul(vp,
                                     lhsT=xT[c][:, kb * 128:(kb + 1) * 128],
                                     rhs=wv[:, c * 128:(c + 1) * 128],
                                     start=(c == 0), stop=(c == ndc - 1))
                # one strided copy per k-block: both heads' 64 value cols
                dst = vb.rearrange("p (k two c) -> p k two c", two=2,
                                   c=128)[:, kb, :, 0:64]
                psum_drain_copy(dst, vp.rearrange("p (two c) -> p two c",
                                                  two=2))
            return emit

        pe_queue.append(("proj", qc, qk_group(wq, qTb)))
        pe_queue.append(("proj", qc, qk_group(wk, kTb)))
        for j in range(4):
            pe_queue.append(("proj", qc, v_group(qc * 4 + j)))

    def queue_outproj(qc):
        def out_group(qb):
            def emit():
                pp = mmps.tile([128, D], f32, name=f"pp_{qb}", tag="mm")
                nc.tensor.matmul(pp,
                                 lhsT=attnT[:, qb * 128:(qb + 1) * 128],
                                 rhs=wp, start=True, stop=True)
                ot = outpool.tile([128, D], bf16, name=f"ot_{qb}", tag="ot")
                psum_drain_copy(ot, pp)
                # Act HWDGE queue so stores don't stall the streaming xT
                # loads; the last chunk alternates queues (SP is free then)
                deng = nc.scalar if (qc < 7 or qb % 2 == 0) else nc.sync
                deng.dma_start(out=out_d[qb * 128:(qb + 1) * 128, :],
                               in_=ot)
            return emit

        for j in range(4):
            pe_queue.append(("out", qc, out_group(qc * 4 + j)))

    vb_k = vb.rearrange("p (k c) -> p k c", c=256)
    maskD_2 = maskD.rearrange("p (two f) -> p two f", two=2)

    def emit_attention_head(qc, h, acc):
        qlo = qc * QCHUNK
        npairs = (qc + 1) * 2  # k-block pairs in causal range
        hsl = slice(h * 64, (h + 1) * 64)
        pend = []  # lagged attnV emission keeps PE fed
        # diagonal pairs FIRST: their masks clear early, off the tail's
        # critical path; the first diagonal block carries start=True and
        # covers the full accumulator width
        kbt_order = [2 * qc, 2 * qc + 1] + list(range(0, 2 * qc))
        kb_first = qc * 4
        kb_last = (qc * 4 - 1) if qc > 0 else (4 * qc + 3)
        for kbt in kbt_order:
            drain_pe(3 if qc < 3 else 2)
            diag_t = kbt - 2 * qc  # 0/1 for the two diagonal pairs, else <0
            # cols < c_lo of both blocks are fully causal-masked: skip
            c_lo = 256 if diag_t == 1 else 0
            ex = expool.tile([128, 1024], i16, name=f"ex{h}_{qc}_{kbt}",
                             tag="ex")
            for j in range(2):
                kb = kbt * 2 + j
                sc = scps.tile([128, 512], f32, name=f"sc{h}_{qc}_{kb}",
                               tag="sc")
                nc.tensor.matmul(
                    sc[:, c_lo:512],
                    lhsT=kTb[hsl, kb * 128:(kb + 1) * 128],
                    rhs=qTb[hsl, qlo + c_lo:qlo + QCHUNK],
                    start=True, stop=True)
                ex_out = ex[:, j * 512 + c_lo:(j + 1) * 512]
                if exp_rr.next() == "A":
                    nc.scalar.activation(ex_out.bitcast(bf16), sc[:, c_lo:512],
                                         EXP, scale=float(SCALE),
                                         bias=nbias[:, 0:1])
                else:
                    nc.vector.tensor_scalar(ex_out, sc[:, c_lo:512],
                                            float(FEXP_A), float(FEXP_B),
                                            MUL, ADD)
            exb = ex.bitcast(bf16)
            ex3 = exb.rearrange("p (two q) -> p two q", two=2)
            if diag_t >= 0:
                # fused causal mask: both diagonal blocks in one op
                c0 = 256 * diag_t
                sl = ex3[:, :, c0:c0 + 256]
                m_eng = nc.vector if mask_rr.next() == "D" else nc.gpsimd
                m_eng.tensor_mul(sl, sl, maskD_2)
            while pend:
                a, kw = pend.pop(0)
                nc.tensor.matmul(a, **kw)
            for j in range(2):
                kb = kbt * 2 + j
                r = kb - qc * 4
                lo = 128 * r if r > 0 else 0  # cols < lo: fully masked
                pend.append((
                    acc[:, lo:QCHUNK],
                    dict(lhsT=vb_k[:, kb, h * 128:(h + 1) * 128],
                         rhs=exb[:, j * 512 + lo:(j + 1) * 512],
                         start=(kb == kb_first), stop=(kb == kb_last))))
        while pend:
            a, kw = pend.pop(0)
            nc.tensor.matmul(a, **kw)

        # normalize deferred into the next head's pair stream; acc rows
        # 64..127 already hold the denominator broadcast over 64 partitions
        def norm_group():
            qs = slice(qlo, qlo + QCHUNK)
            if normalize:
                rf64 = recpool.tile([64, QCHUNK], f32, name=f"rf{h}_{qc}",
                                    tag="rf")
                nc.vector.reciprocal(rf64, acc[64:128, :])
                nc.vector.tensor_mul(attnT[hsl, qs], acc[0:64, :], rf64)
            else:
                nc.vector.tensor_copy(attnT[hsl, qs], acc[0:64, :])
        pe_queue.insert(0, ("norm", qc, norm_group))

    def emit_attention(qc):
        drain_proj_through(qc)
        for h in range(2):
            # single shared acc bank: h1 reuses h0's buffer after its norm
            acc = accps.tile([128, QCHUNK], f32, name=f"acc{h}_{qc}",
                             tag="acc")
            emit_attention_head(qc, h, acc)

    queue_proj(0)
    drain_pe(6)          # chunk 0 projections up-front
    for qc in range(nqc):
        if qc + 1 < nqc:
            queue_proj(qc + 1)
        emit_attention(qc)
        queue_outproj(qc)
    drain_pe(len(pe_queue))


def _in_maps(x, W_q, W_k, W_v, W_proj):
    import ml_dtypes
    bf = ml_dtypes.bfloat16
    maps = []
    for c in range(8):
        b, hp = c // 4, c % 4
        cols = slice(hp * 128, (hp + 1) * 128)
        maps.append({
            "xT": np.ascontiguousarray(x[b].T).astype(bf),
            "wqT": np.ascontiguousarray(W_q.T[:, cols]).astype(bf),
            "wkT": np.ascontiguousarray(W_k.T[:, cols]).astype(bf),
            "wvT": np.ascontiguousarray(W_v.T[:, cols]).astype(bf),
            "wpT": np.ascontiguousarray(W_proj[:, cols].T).astype(bf),
        })
    return maps


def kernel(x, W_q, W_k, W_v, W_proj, b_proj, _trace=False):
    from concourse.bass_utils import run_bass_kernel_spmd

    x = np.asarray(x, dtype=np.float32)
    W_q = np.asarray(W_q, dtype=np.float32)
    W_k = np.asarray(W_k, dtype=np.float32)
    W_v = np.asarray(W_v, dtype=np.float32)
    W_proj = np.asarray(W_proj, dtype=np.float32)
    b_proj = np.asarray(b_proj, dtype=np.float32)

    if "nc" not in _CACHE:
        _CACHE["nc"] = _build()
    nc = _CACHE["nc"]

    res = run_bass_kernel_spmd(nc, _in_maps(x, W_q, W_k, W_v, W_proj),
                               core_ids=list(range(8)), trace=_trace)
    out = np.empty((B, S, D), dtype=np.float32)
    for b in range(B):
        acc = res.results[4 * b]["out_p"].astype(np.float32)
        for j in range(1, 4):
            acc = acc + res.results[4 * b + j]["out_p"].astype(np.float32)
        out[b] = acc + b_proj
    if _trace:
        _CACHE["last_trace"] = res
    return out
